# revision 20
# baseline (speedup 1.0000x reference)
"""Trainium2 Bass kernel for nn_Attention_29935922053658 (sparse frame attention).

Sharding: data-parallel over batch B=8 -> 8 NeuronCores (1 batch each).
Per-core: fused qkv-proj + frame-local attention (196-token frames, cls token
attends globally) + out-proj, bf16 matmuls / fp32 accum.

Structure (v2): software-pipelined frame pairs. Per pair p, the projection
stage A(p+1) (x cast/transpose, qk/v projections) is emitted interleaved into
the attention stage B(p) so the PE queue never drains (keeps the PE p-state at
full clock). Frame normalization+out-proj are deferred one frame so the
softmax-denominator chain (act-engine copies -> reciprocal) hides behind PE
work. CLS attention accumulates in two dedicated PSUM banks across the whole
kernel via open accumulation groups; k/v stay resident in SBUF for it.
"""

import sys
import types
import json

for _p in ("/opt/trn_rl_repo", "/root/.axon_site"):
    if _p not in sys.path:
        sys.path.insert(0, _p)

import numpy as np

# ---------------------------------------------------------------------------
# Environment shims (required under the axon-proxied PJRT runtime):
#  1. antenv.axon_hooks registry (missing in this image) so trace=True can work.
#  2. Split >2 sync-waits off instructions — this walrus build's CoreV3
#     codegen rejects them ("Too many sync wait commands").
#  3. upload_artifacts: no artifact bucket in this container.
# ---------------------------------------------------------------------------


def _install_shims():
    import antenv

    if "antenv.axon_hooks" not in sys.modules:
        m = types.ModuleType("antenv.axon_hooks")
        m._hook = None

        def set_axon_ntff_profile_hook(h):
            m._hook = h

        def get_axon_ntff_profile_hook():
            return m._hook

        m.set_axon_ntff_profile_hook = set_axon_ntff_profile_hook
        m.get_axon_ntff_profile_hook = get_axon_ntff_profile_hook
        sys.modules["antenv.axon_hooks"] = m
        antenv.axon_hooks = m
        try:
            from trn_agent_boot.trn_boot import _ntff_profile_via_ctypes

            hook = _ntff_profile_via_ctypes("/opt/axon/libaxon_pjrt.so")
            if hook is not None:
                m._hook = hook
        except Exception:
            pass

    import concourse.bass_utils as bu
    import concourse.bass2jax as b2j

    if not getattr(bu, "_drain_patch_installed", False):
        bu._drain_patch_installed = True
        bu.upload_artifacts = lambda tmpdir: "local://" + str(tmpdir)

        _orig = b2j.compile_bir_kernel

        def _patched_compile(ant_bir_str, compile_dir, neff_name="file.neff"):
            # This walrus build's codegen accepts at most ONE sync-wait per
            # instruction; hoist extras onto chained same-engine NoOps.
            d = json.loads(ant_bir_str)
            changed = False
            for fn in d.get("functions", []):
                for blk in fn.get("blocks", []):
                    insts = blk.get("instructions", [])
                    out = []
                    for ins in insts:
                        si = ins.get("sync_info") or {}
                        waits = si.get("on_wait") or []
                        if len(waits) > 1:
                            for ci, w in enumerate(waits[:-1]):
                                out.append(
                                    {
                                        "debug": ins.get("debug", 0),
                                        "engine": ins["engine"],
                                        "ins": [],
                                        "outs": [],
                                        "name": ins["name"] + f"-ws{ci}",
                                        "opcode": "NoOp",
                                        "sync_info": {
                                            "on_update": [],
                                            "on_wait": [w],
                                        },
                                    }
                                )
                            si["on_wait"] = waits[-1:]
                            changed = True
                        out.append(ins)
                    blk["instructions"] = out
            if changed:
                ant_bir_str = json.dumps(d).encode()
            return _orig(ant_bir_str, compile_dir, neff_name=neff_name)

        b2j.compile_bir_kernel = _patched_compile


_install_shims()

import concourse.bass as bass
import concourse.mybir as mybir
import concourse.tile as tile
from concourse.bass_utils import run_bass_kernel_spmd

f32 = mybir.dt.float32
bf16 = mybir.dt.bfloat16
AF = mybir.ActivationFunctionType

# Problem constants (hardcoded per spec)
N_SEQ = 3137
DIM = 512
H = 8
DH = 64
F = 16
NF = 196  # tokens per frame
NK = 197  # keys per frame block (frame + cls)
N_CORES = 8
P = 8  # frame pairs

TOK_CHUNKS = [(0, 128), (128, 68)]


def build_kernel():
    nc = bass.Bass()
    x_d = nc.dram_tensor("x", [N_SEQ, DIM], f32, kind="ExternalInput")
    wqkv_d = nc.dram_tensor("wqkv", [DIM, 3 * DIM], f32, kind="ExternalInput")
    wout_d = nc.dram_tensor("wout", [DIM, DIM], f32, kind="ExternalInput")
    bout_d = nc.dram_tensor("bout", [1, DIM], f32, kind="ExternalInput")
    ident_d = nc.dram_tensor("ident", [128, 128], bf16, kind="ExternalInput")
    ind8_d = nc.dram_tensor("ind8", [8, DIM], bf16, kind="ExternalInput")
    out_d = nc.dram_tensor("out", [N_SEQ, DIM], f32, kind="ExternalOutput")

    with tile.TileContext(nc) as tc:
        with (
            tc.tile_pool(name="const", bufs=1) as cpool,
            tc.tile_pool(name="work", bufs=2) as wpool,
            tc.tile_pool(name="at_sb", bufs=3) as apool,
            tc.tile_pool(name="qk_ps", bufs=2, space="PSUM") as qkps,
            tc.tile_pool(name="at_ps", bufs=2, space="PSUM") as atps,
            tc.tile_pool(name="po_ps", bufs=2, space="PSUM") as pops,
            tc.tile_pool(name="cls_ps", bufs=1, space="PSUM") as clsps,
        ):
            # ============ persistent SBUF state ============
            # k transposed, all pairs: kT_big[c][:, p*2NK + fl*NK + {0..195,196cls}]
            kT_big = [
                cpool.tile([128, P * 2 * NK], bf16, name=f"kT{c}", tag=f"kT{c}")
                for c in range(4)
            ]
            # v natural + ones col, all (frame, chunk): chunk ch=2g+t at cols
            # ch*520, per-head layout (h: 64 v-dims + ones)
            v_big = cpool.tile([128, 32 * 520], bf16, name="v_big", tag="v_big")
            nc.gpsimd.memset(
                v_big[:].rearrange("p (a c) -> p a c", c=65)[:, :, 64:65], 1.0
            )

            # cls numerator/denominator accumulators (open accumulation groups
            # for the whole kernel): heads 0-3 in A, 4-7 in B, 65 cols per head
            cls_psA = clsps.tile([8, 260], f32, name="cls_psA", tag="clsA")
            cls_psB = clsps.tile([8, 260], f32, name="cls_psB", tag="clsB")

            # ---------------- A-stage machinery ----------------
            x32_tiles = {}  # pair -> list of 4 x32 tiles
            xbf_tiles = {}  # pair -> list of 4 bf16 tiles
            xT_tiles = {}  # pair -> list of 4 [128, 392]
            qT_tiles = {}  # pair -> list of 4 [128, 392] (q m-chunks)

            def emit_x_loads(p):
                tiles = []
                pr0 = 1 + p * 2 * NF
                for fl in range(2):
                    for t, (t0, tn) in enumerate(TOK_CHUNKS):
                        i = 2 * fl + t
                        x32 = wpool.tile(
                            [tn, DIM], f32, name=f"x32_{i}", tag=f"x32_{i}"
                        )
                        nc.sync.dma_start(
                            out=x32[:],
                            in_=x_d[pr0 + fl * NF + t0 : pr0 + fl * NF + t0 + tn, :],
                        )
                        tiles.append(x32)
                x32_tiles[p] = tiles

            def emit_x_cast(p, i, eng=None):
                # f32 -> bf16 (SBUF->SBUF), gpsimd by default; padded rows
                t = i % 2
                tn = TOK_CHUNKS[t][1]
                pt = 128 if t == 0 else 80
                xb = wpool.tile([pt, DIM], bf16, name=f"xbf_{i}", tag=f"xbf_{i}")
                if t == 1:
                    nc.gpsimd.memset(xb[64:80, :], 0.0)
                if eng == "v":
                    nc.vector.tensor_copy(xb[0:tn, :], x32_tiles[p][i][:])
                elif eng == "s":
                    nc.scalar.copy(xb[0:tn, :], x32_tiles[p][i][:])
                else:
                    nc.gpsimd.tensor_copy(xb[0:tn, :], x32_tiles[p][i][:])
                xbf_tiles.setdefault(p, [None] * 4)[i] = xb

            def emit_trans(p, c):
                ps_t = qkps.tile([128, 2 * NF], bf16, name="ps_t", tag="qk")
                for fl in range(2):
                    for t, (t0, tn) in enumerate(TOK_CHUNKS):
                        g0 = fl * NF + t0
                        nc.tensor.transpose(
                            ps_t[:, g0 : g0 + tn],
                            xbf_tiles[p][2 * fl + t][0:tn, c * 128 : (c + 1) * 128],
                            ident[0:tn, 0:tn],
                        )
                xt = wpool.tile([128, 2 * NF], bf16, name=f"xT_{c}", tag=f"xT_{c}")
                nc.vector.tensor_copy(xt[:], ps_t[:])
                xT_tiles.setdefault(p, [None] * 4)[c] = xt

            def emit_qkproj(p, m, qkT_cls):
                ps_p = qkps.tile([128, 2 * NF], f32, name="ps_p", tag="qk")
                for c in range(4):
                    nc.tensor.matmul(
                        ps_p[:],
                        lhsT=wqkv_bf[c][:, m * 128 : (m + 1) * 128],
                        rhs=xT_tiles[p][c][:],
                        start=(c == 0),
                        stop=(c == 3),
                    )
                if m < 4:
                    qt = wpool.tile(
                        [128, 2 * NF], bf16, name=f"qT_{m}", tag=f"qT_{m}"
                    )
                    nc.vector.tensor_copy(qt[:], ps_p[:])
                    qT_tiles.setdefault(p, [None] * 4)[m] = qt
                else:
                    c2 = m - 4
                    kb = p * 2 * NK
                    nc.vector.tensor_copy(
                        kT_big[c2][:, kb : kb + 2 * NK].rearrange(
                            "p (f k) -> p f k", k=NK
                        )[:, :, 0:NF],
                        ps_p[:].rearrange("p (f k) -> p f k", k=NF),
                    )
                    nc.gpsimd.tensor_copy(
                        kT_big[c2][:, kb + NF : kb + NF + 1], qkT_cls[m][:]
                    )
                    nc.gpsimd.tensor_copy(
                        kT_big[c2][:, kb + NK + NF : kb + NK + NF + 1], qkT_cls[m][:]
                    )

            def emit_vproj(p, fl, t, v_ext_cls):
                t0, tn = TOK_CHUNKS[t]
                ch = (2 * p + fl) * 2 + t
                ps_v = qkps.tile([tn, DIM], f32, name="ps_v", tag="qk")
                for c in range(4):
                    nc.tensor.matmul(
                        ps_v[:],
                        lhsT=xT_tiles[p][c][:, fl * NF + t0 : fl * NF + t0 + tn],
                        rhs=wqkv_bf[c][:, 2 * DIM : 3 * DIM],
                        start=(c == 0),
                        stop=(c == 3),
                    )
                dst = v_big[:, ch * 520 : (ch + 1) * 520]
                if t == 0:
                    nc.scalar.copy(
                        dst[0:tn, :].rearrange("p (h c) -> p h c", c=65)[:, :, 0:64],
                        ps_v[:].rearrange("p (h c) -> p h c", c=64),
                    )
                else:
                    nc.vector.tensor_copy(
                        dst[0:tn, :].rearrange("p (h c) -> p h c", c=65)[:, :, 0:64],
                        ps_v[:].rearrange("p (h c) -> p h c", c=64),
                    )
                if t == 1:
                    nc.sync.dma_start(out=dst[68:69, :], in_=v_ext_cls[:])

            # ============ preamble: loads first, weights streamed behind ====
            emit_x_loads(0)
            ident = cpool.tile([128, 128], bf16, name="ident", tag="ident")
            nc.sync.dma_start(out=ident[:], in_=ident_d[:])
            xcls32 = []
            for c in range(4):
                t32 = wpool.tile([128, 1], f32, name="xclsld", tag="xclsld", bufs=4)
                nc.sync.dma_start(
                    out=t32[:],
                    in_=x_d[0:1, c * 128 : (c + 1) * 128].rearrange("a b -> b a"),
                )
                xcls32.append(t32)
            ind8 = cpool.tile([8, DIM], bf16, name="ind8", tag="ind8")
            nc.sync.dma_start(out=ind8[:], in_=ind8_d[:])

            # wqkv streamed in halves: subtile deps let the first qk
            # projections start as soon as the first half of each chunk lands
            wqkv_bf = []
            wld = []
            for c in range(4):
                t32 = wpool.tile([128, 3 * DIM], f32, name="wld", tag="wld", bufs=4)
                for hh in range(2):
                    sl = slice(hh * 768, (hh + 1) * 768)
                    nc.sync.dma_start(
                        out=t32[:, sl], in_=wqkv_d[c * 128 : (c + 1) * 128, sl]
                    )
                wld.append(t32)
            for hh in range(2):
                sl = slice(hh * 768, (hh + 1) * 768)
                for c in range(4):
                    if hh == 0:
                        tb = cpool.tile(
                            [128, 3 * DIM], bf16, name=f"wqkv{c}", tag=f"wqkv{c}"
                        )
                        wqkv_bf.append(tb)
                    if c % 2 == 0:
                        nc.vector.tensor_copy(wqkv_bf[c][:, sl], wld[c][:, sl])
                    else:
                        nc.scalar.copy(wqkv_bf[c][:, sl], wld[c][:, sl])

            xT_cls = []
            for c in range(4):
                tb = cpool.tile([128, 1], bf16, name=f"xTcls{c}", tag=f"xTcls{c}")
                nc.vector.tensor_copy(tb[:], xcls32[c][:])
                xT_cls.append(tb)

            # pair 0: spread casts across engines so transposes start early
            for i, eng in enumerate(("v", "s", None, "v")):
                emit_x_cast(0, i, eng)
            for c in range(4):
                emit_trans(0, c)

            # ============ preamble: cls state ============
            # qkT_cls[m]: [128,1] bf16 (q chunks 0-3, k chunks 4-7)
            qkT_cls = []
            for m in range(8):
                ps = atps.tile([128, 1], f32, name="ps_qc", tag="at")
                for c in range(4):
                    nc.tensor.matmul(
                        ps[:],
                        lhsT=wqkv_bf[c][:, m * 128 : (m + 1) * 128],
                        rhs=xT_cls[c][:],
                        start=(c == 0),
                        stop=(c == 3),
                    )
                tb = cpool.tile([128, 1], bf16, name=f"qkTcls{m}", tag=f"qkTcls{m}")
                nc.vector.tensor_copy(tb[:], ps[:])
                qkT_cls.append(tb)

            # qblk[c]: [128, 8] bf16 block-diagonal cls query
            qblk = []
            for c in range(4):
                tb = cpool.tile([128, 8], bf16, name=f"qblk{c}", tag=f"qblk{c}")
                nc.gpsimd.memset(tb[:], 0.0)
                nc.vector.tensor_copy(tb[0:64, 2 * c : 2 * c + 1], qkT_cls[c][0:64, :])
                nc.vector.tensor_copy(
                    tb[64:128, 2 * c + 1 : 2 * c + 2], qkT_cls[c][64:128, :]
                )
                qblk.append(tb)

            # v_ext_cls [1, 520] bf16: cls v row + per-head ones columns
            # (needs qkv_cls v-part: one [1,512] projection)
            ps_vc = atps.tile([1, DIM], f32, name="ps_vc", tag="at")
            for c in range(4):
                nc.tensor.matmul(
                    ps_vc[:],
                    lhsT=xT_cls[c][:],
                    rhs=wqkv_bf[c][:, 2 * DIM : 3 * DIM],
                    start=(c == 0),
                    stop=(c == 3),
                )
            v_ext_cls = cpool.tile([1, 520], bf16, name="v_ext_cls", tag="v_ext_cls")
            nc.gpsimd.memset(
                v_ext_cls[:].rearrange("p (h c) -> p h c", c=65)[:, :, 64:65], 1.0
            )
            nc.vector.tensor_copy(
                v_ext_cls[:].rearrange("p (h c) -> p h c", c=65)[:, :, 0:64],
                ps_vc[:].rearrange("p (h c) -> p h c", c=64),
            )

            # cls self-term opens the cls accumulation groups (start=True)
            ps_s0 = atps.tile([1, 8], f32, name="ps_s0", tag="at")
            for c in range(4):
                nc.tensor.matmul(
                    ps_s0[:],
                    lhsT=qkT_cls[4 + c][:],
                    rhs=qblk[c][:],
                    start=(c == 0),
                    stop=(c == 3),
                )
            aT_self = wpool.tile([1, 8], bf16, name="aT_self", tag="aT_self")
            nc.scalar.activation(aT_self[:], ps_s0[:], AF.Exp)
            for nch, ps_n in enumerate((cls_psA, cls_psB)):
                nc.tensor.matmul(
                    ps_n[:],
                    lhsT=aT_self[:],
                    rhs=v_ext_cls[:, nch * 260 : (nch + 1) * 260],
                    start=True,
                    stop=False,
                    skip_group_check=True,
                )

            # ============ finish A(0), weights tail ============
            for m in range(8):
                emit_qkproj(0, m, qkT_cls)
            for fl in range(2):
                for t in range(2):
                    emit_vproj(0, fl, t, v_ext_cls)

            wout_bf = []
            for c in range(4):
                t32 = wpool.tile([128, DIM], f32, name="wld2", tag="wld2")
                nc.sync.dma_start(out=t32[:], in_=wout_d[c * 128 : (c + 1) * 128, :])
                tb = cpool.tile([128, DIM], bf16, name=f"wout{c}", tag=f"wout{c}")
                nc.vector.tensor_copy(tb[:], t32[:])
                wout_bf.append(tb)

            # bias broadcast to 128 partitions via rank-1 matmul
            bout_sb = cpool.tile([1, DIM], f32, name="bout", tag="bout")
            nc.sync.dma_start(out=bout_sb[:], in_=bout_d[:])
            ones_row = cpool.tile([1, 128], f32, name="ones_row", tag="ones_row")
            nc.gpsimd.memset(ones_row[:], 1.0)
            ps_b = pops.tile([128, DIM], f32, name="ps_b", tag="po")
            nc.tensor.matmul(ps_b[:], lhsT=ones_row[:], rhs=bout_sb[:], start=True, stop=True)
            bout_bc = cpool.tile([128, DIM], f32, name="bout_bc", tag="bout_bc")
            nc.vector.tensor_copy(bout_bc[:], ps_b[:])

            # ---------------- B-stage machinery ----------------
            s8_tiles = {}
            sc_tiles = {}
            attnT_tiles = {}

            def emit_cls_chunk_sim(g, t):
                # cls attention logits of frame chunk (g, t): frame keys only
                # (cls self-term handled in preamble)
                p, fl = g // 2, g % 2
                t0, tn = TOK_CHUNKS[t]
                kcol = p * 2 * NK + fl * NK + t0
                ps_c = atps.tile([tn, 8], f32, name="ps_c", tag="at")
                for c in range(4):
                    nc.tensor.matmul(
                        ps_c[:],
                        lhsT=kT_big[c][:, kcol : kcol + tn],
                        rhs=qblk[c][:],
                        start=(c == 0),
                        stop=(c == 3),
                    )
                a_cls = apool.tile([tn, 8], bf16, name="a_cls", tag="a_cls")
                nc.scalar.activation(a_cls[:], ps_c[:], AF.Exp)
                return a_cls

            def emit_cls_chunk_nums(g, t, a_cls, last):
                t0, tn = TOK_CHUNKS[t]
                ch = 2 * g + t
                for nch, ps_n in enumerate((cls_psA, cls_psB)):
                    nc.tensor.matmul(
                        ps_n[:],
                        lhsT=a_cls[:],
                        rhs=v_big[0:tn, ch * 520 + nch * 260 : ch * 520 + (nch + 1) * 260],
                        start=False,
                        stop=last,
                        skip_group_check=True,
                    )

            def emit_sim(g, h):
                # qk^T for head h -> exp into aT (returned for the av stage)
                p, fl = g // 2, g % 2
                kc, r = h // 2, (h % 2) * 64
                kb = p * 2 * NK + fl * NK
                qb = fl * NF
                ps_s = atps.tile([128, 2 * NF], f32, name="ps_s", tag="at")
                nc.tensor.matmul(
                    ps_s[:, 0:NF],
                    lhsT=kT_big[kc][r : r + 64, kb : kb + 128],
                    rhs=qT_tiles[p][kc][r : r + 64, qb : qb + NF],
                    start=True,
                    stop=True,
                )
                nc.tensor.matmul(
                    ps_s[0:69, NF : 2 * NF],
                    lhsT=kT_big[kc][r : r + 64, kb + 128 : kb + NK],
                    rhs=qT_tiles[p][kc][r : r + 64, qb : qb + NF],
                    start=True,
                    stop=True,
                )
                aT = apool.tile([128, 2 * NF], bf16, name="aT", tag="aT")
                nc.scalar.activation(aT[:], ps_s[:], AF.Exp)
                return aT

            def emit_av(g, h, aT):
                kc, r = h // 2, (h % 2) * 64
                ch0, ch1 = 2 * g, 2 * g + 1
                po = pops.tile([65, NF], f32, name="po", tag="po")
                nc.tensor.matmul(
                    po[:],
                    lhsT=v_big[0:128, ch0 * 520 + h * 65 : ch0 * 520 + (h + 1) * 65],
                    rhs=aT[:, 0:NF],
                    start=True,
                    stop=False,
                )
                nc.tensor.matmul(
                    po[:],
                    lhsT=v_big[0:69, ch1 * 520 + h * 65 : ch1 * 520 + (h + 1) * 65],
                    rhs=aT[0:69, NF : 2 * NF],
                    start=False,
                    stop=True,
                )
                nc.scalar.copy(
                    sc_tiles[g][0:1, h * NF : (h + 1) * NF], po[64:65, :]
                )
                if h % 4 == 3:
                    nc.scalar.copy(
                        attnT_tiles[g][h // 2][r : r + 64, :], po[0:64, :]
                    )
                else:
                    nc.vector.tensor_copy(
                        attnT_tiles[g][h // 2][r : r + 64, :], po[0:64, :]
                    )

            def emit_frame_heads(g, thunks):
                sc_tiles[g] = wpool.tile(
                    [1, 8 * NF], f32, name="sc_all", tag="sc_all", bufs=3
                )
                attnT_tiles[g] = [
                    wpool.tile([128, NF], bf16, name=f"attnT_{c}", tag=f"attnT_{c}")
                    for c in range(4)
                ]
                # cls sims+exps up front; their nums matmuls deferred to slot 1
                cls_nums = [
                    emit_cls_chunk_sim(g, 0),
                    emit_cls_chunk_sim(g, 1),
                ]
                # head-slot pipeline: slot h issues sim(h)+exp(h) then av(h-1),
                # so the PE never waits on exp within a slot
                aT_prev = None
                for h in range(8):
                    aT_cur = emit_sim(g, h)
                    if h == 1:
                        emit_cls_chunk_nums(g, 0, cls_nums[0], last=False)
                        emit_cls_chunk_nums(g, 1, cls_nums[1], last=(g == 15))
                    if aT_prev is not None:
                        emit_av(g, h - 1, aT_prev)
                    aT_prev = aT_cur
                    if thunks:
                        thunks.pop(0)()
                emit_av(g, 7, aT_prev)
                # split denominators across partitions (latency hidden: the
                # tail consuming s8 is emitted a frame later)
                s8_tiles[g] = wpool.tile([8, NF], f32, name="s8", tag="s8", bufs=3)
                nc.sync.dma_start(out=s8_tiles[g][:], in_=sc_tiles[g][0:1, :])

            def emit_frame_tail(g):
                s8 = s8_tiles.pop(g)
                attnT = attnT_tiles.pop(g)
                nc.vector.reciprocal(s8[:], s8[:])
                rs8 = wpool.tile([8, NF], bf16, name="rs8", tag="rs8", bufs=3)
                nc.vector.tensor_copy(rs8[:], s8[:])
                for c in range(4):
                    ps_r = atps.tile([128, NF], f32, name="ps_r", tag="at")
                    nc.tensor.matmul(
                        ps_r[:],
                        lhsT=ind8[:, c * 128 : (c + 1) * 128],
                        rhs=rs8[:],
                        start=True,
                        stop=True,
                    )
                    nc.vector.tensor_mul(attnT[c][:], attnT[c][:], ps_r[:])
                r0 = 1 + g * NF
                for t, (t0, tn) in enumerate(TOK_CHUNKS):
                    ps_o = pops.tile([tn, DIM], f32, name="ps_o", tag="po")
                    for c in range(4):
                        nc.tensor.matmul(
                            ps_o[:],
                            lhsT=attnT[c][:, t0 : t0 + tn],
                            rhs=wout_bf[c][:],
                            start=(c == 0),
                            stop=(c == 3),
                        )
                    o_sb = wpool.tile([tn, DIM], f32, name=f"osb_{t}", tag=f"osb_{t}")
                    nc.vector.tensor_add(o_sb[:], ps_o[:], bout_bc[0:tn, :])
                    nc.sync.dma_start(
                        out=out_d[r0 + t0 : r0 + t0 + tn, :], in_=o_sb[:]
                    )

            def a_thunks(p):
                th = []
                if p < P:
                    th.append(lambda p=p: emit_x_loads(p))
                    for i in range(4):
                        th.append(lambda p=p, i=i: emit_x_cast(p, i))
                    for c in range(4):
                        th.append(lambda p=p, c=c: emit_trans(p, c))
                    for m in range(8):
                        th.append(lambda p=p, m=m: emit_qkproj(p, m, qkT_cls))
                    for fl in range(2):
                        for t in range(2):
                            th.append(
                                lambda p=p, fl=fl, t=t: emit_vproj(p, fl, t, v_ext_cls)
                            )
                return th

            # ============ main pipelined loop ============
            # tail(g) is emitted after heads(g+1): a full frame of PE work
            # separates a frame's denominator chain from its consumption
            for p in range(P):
                thunks = a_thunks(p + 1)
                emit_frame_heads(2 * p, thunks)
                if p > 0:
                    emit_frame_tail(2 * p - 1)
                emit_frame_heads(2 * p + 1, thunks)
                while thunks:
                    thunks.pop(0)()
                emit_frame_tail(2 * p)

            # cls extraction overlaps the two remaining frame tails (the cls
            # accumulation groups closed during heads(15))
            # [8, 520] = heads 0-3 cols 0:260 (psA), heads 4-7 cols 260:520
            # (psB) -> per-head 65-block of head h starts at col h*65
            cls_sb = wpool.tile([8, 520], f32, name="cls_sb", tag="cls_sb")
            nc.vector.tensor_copy(cls_sb[:, 0:260], cls_psA[:])
            nc.vector.tensor_copy(cls_sb[:, 260:520], cls_psB[:])
            diag_sb = wpool.tile([8, 65], f32, name="diag", tag="diag")
            for h in range(8):
                eng = nc.sync if h % 2 == 0 else nc.scalar
                eng.dma_start(
                    out=diag_sb[h : h + 1, :],
                    in_=cls_sb[h : h + 1, h * 65 : (h + 1) * 65],
                )
            emit_frame_tail(15)

            # ============ cls epilogue ============
            rden = wpool.tile([8, 1], f32, name="rden", tag="rden")
            nc.vector.reciprocal(rden[:], diag_sb[:, 64:65])
            cls_n = wpool.tile([8, 64], bf16, name="cls_n", tag="cls_n")
            nc.vector.tensor_scalar_mul(cls_n[:], diag_sb[:, 0:64], rden[:, 0:1])
            ps_t = atps.tile([64, 8], bf16, name="ps_tc", tag="at")
            nc.tensor.transpose(ps_t[:], cls_n[:], ident[0:8, 0:8])
            attnT_cls = [
                wpool.tile([128, 1], bf16, name=f"aTc{c}", tag=f"aTc{c}")
                for c in range(4)
            ]
            for h in range(8):
                nc.vector.tensor_copy(
                    attnT_cls[h // 2][(h % 2) * 64 : (h % 2) * 64 + 64, :],
                    ps_t[:, h : h + 1],
                )
            ps_oc = pops.tile([1, DIM], f32, name="ps_oc", tag="po")
            for c in range(4):
                nc.tensor.matmul(
                    ps_oc[:],
                    lhsT=attnT_cls[c][:],
                    rhs=wout_bf[c][:],
                    start=(c == 0),
                    stop=(c == 3),
                )
            o_cls = wpool.tile([1, DIM], f32, name="o_cls", tag="o_cls")
            nc.vector.tensor_add(o_cls[:], ps_oc[:], bout_bc[0:1, :])
            nc.sync.dma_start(out=out_d[0:1, :], in_=o_cls[:])

    return nc


_NC_CACHE = {}


def _get_nc():
    if "nc" not in _NC_CACHE:
        _NC_CACHE["nc"] = build_kernel()
    return _NC_CACHE["nc"]


def kernel(x, Wqkv, Wout, bout, f, _trace=False, _trace_kwargs=None):
    assert int(f) == F, f"kernel hardcoded for f={F}, got {f}"
    x = np.asarray(x, np.float32)
    Wqkv_s = np.asarray(Wqkv, np.float32).copy()
    Wqkv_s[:, :DIM] *= DH ** -0.5  # fold q scaling into the projection
    Wout = np.asarray(Wout, np.float32)
    bout2 = np.asarray(bout, np.float32).reshape(1, DIM)

    import ml_dtypes

    ident_np = np.eye(128, dtype=ml_dtypes.bfloat16)
    ind8_np = np.zeros((8, DIM), dtype=ml_dtypes.bfloat16)
    for k in range(8):
        ind8_np[k, k * 64 : (k + 1) * 64] = 1.0

    nc = _get_nc()
    in_maps = [
        {
            "x": x[i],
            "wqkv": Wqkv_s,
            "wout": Wout,
            "bout": bout2,
            "ident": ident_np,
            "ind8": ind8_np,
        }
        for i in range(N_CORES)
    ]
    res = run_bass_kernel_spmd(
        nc,
        in_maps,
        list(range(N_CORES)),
        trace=_trace,
        **(_trace_kwargs or {}),
    )
    out = np.stack([res.results[i]["out"] for i in range(N_CORES)], axis=0)
    if _trace:
        kernel.last_results = res
    return out
